# revision 23
# baseline (speedup 1.0000x reference)
"""Trainium2 Bass kernel for nn_CrossConvLayerV2 (gnn_message_passing).

Math (reference):
    coords = points[..., :3]; feats = points[..., 3:]          # [B,n,3], [B,n,f]
    probes[b,l,m] = centers[b,l] + PROBES[m]                    # [B,l,m,3]
    sq[b,l,m,n]  = ||coords[b,n] - probes[b,l,m]||^2
    kern         = C / (sq + C)          (C = 0.1)
    agg[b,l,m,f] = (1/n) sum_n kern * feats
    out[b,l,:]   = agg.reshape(l, m*f) @ W + bias               # [B,l,256]

Strategy (v2):
  - Shard centers dim l (256) over 8 cores -> 32 centers/core, zero
    communication; the host gathers the 8 [B,32,256] shards.
  - u = 10*sq + 1 via ONE fp8-e4m3 DoubleRow matmul (0.5 cyc/row, 2x
    the bf16 stream rate). The expansion u = q_n + r_col + sum_k c_k*t_k
    is split into 55 fp8 rows (5-piece sequential e4m3 splits with
    escalating power-of-2 scales; cross terms keep piece pairs with
    i+j<=4; per-row power-of-2 balance scales). fp8 x fp8 products are
    exact in fp32 PSUM accumulation, so u is accurate to ~4e-3 rel.
  - kern = 1/u: ACT Reciprocal LUT (~1.2e-5 rel) for most chunks, DVE
    exact reciprocal for t%8 < RECIP_DVE_OF8 chunks (engine balancing);
    written as fp16.
  - agg[f, (m,l')] += feats_chunk^T @ kern^T  (fp16, PSUM accumulate).
  - The PE instruction stream is software-pipelined with a skew so agg
    matmuls never stall on the reciprocal -> the PE p-state ramps to
    full clock (gaps reset the DVFS ramp).
  - Weighter: agg -> single bf16 piece; W/n in two bf16 pieces (hi+lo);
    52 small matmuls. b_weighter added on host (zeros here).
  - Walrus: at most ONE semaphore wait per instruction; a post-build
    pass splits multi-wait instructions into single-wait NoOp carriers.
"""

import sys

sys.path.insert(0, "/opt/trn_rl_repo")

import numpy as np
import ml_dtypes

# ---- problem constants (hardcoded per contract) ----
B, N, L, D, F = 2, 4096, 256, 3, 16
M = 26
OUT_D = 256
COEFF = 0.1
DIST = 3.0
N_CORES = 8
L_LOC = L // N_CORES          # 32 centers per core
N_SLABS = 2                   # jobs per batch elem per core
L_SLAB = L_LOC // N_SLABS     # 16 centers per job
JM = M * L_SLAB               # 416 = free dim of kern^T tiles
N_JOBS = B * N_SLABS          # 4 jobs per core
NT = N // 128                 # 32 n-chunks

N_PIECES = 5                  # e4m3 pieces per scalar quantity
MAX_IJ = 4                    # cross-term piece pairs kept: i+j <= MAX_IJ
N_ROWS = 2 * N_PIECES + 3 * sum(1 for i in range(N_PIECES) for j in range(N_PIECES) if i + j <= MAX_IJ)
# contraction rows padded to 128: the PE streams at FULL rate (1 col/cycle
# @2.4GHz) only when the stationary spans 128 partitions; small-K matmuls
# run at half rate. Zero rows are exact filler.
KROWS = 128

# of every 7 chunk-PAIRS, how many run the reciprocal on DVE (exact
# iterative divide, ~6 cycles/elem — ~6x ACT's LUT, but a parallel engine)
RECIP_DVE_OF7 = 0
# PE software-pipeline skew in PAIRS: aggs of pair u issue after sqs of pair u+SKEW
SKEW = 3

E4 = ml_dtypes.float8_e4m3


def _make_probes() -> np.ndarray:
    angles = np.array(
        [[j * 0.125 - 0.125, i * 0.125 + (j - 1) * 0.0625] for j in range(3) for i in range(8)]
        + [[-0.25, 0.0], [0.25, 0.0]],
        dtype=np.float64,
    ) * (2.0 * np.pi)
    a, b = angles[:, 0], angles[:, 1]
    pts = np.stack([np.sin(a), np.cos(a) * np.cos(b), np.cos(a) * np.sin(b)], axis=-1) * DIST
    return pts.astype(np.float32)  # [26, 3]


PROBES = _make_probes()

_NC = None
_NC_KEY = None


def _act_reciprocal(nc, out_ap, in_ap):
    """nc.scalar.activation(func=Reciprocal) minus the library guard.
    out = 1/in_ on the ACT engine (LUT path; measured ~1.2e-5 rel here)."""
    import concourse.mybir as mybir

    eng = nc.scalar
    inputs = [eng.lower_ap(in_ap)]
    for val in (0.0, 1.0, 0.0):  # bias, scale, alpha — immediates
        inputs.append(mybir.ImmediateValue(dtype=mybir.dt.float32, value=val))
    return eng.add_instruction(
        mybir.InstActivation(
            name=nc.get_next_instruction_name(),
            func=mybir.ActivationFunctionType.Reciprocal,
            ins=inputs,
            outs=[eng.lower_ap(out_ap)],
        )
    )


def _split_multi_waits(nc):
    """This walrus build encodes at most ONE semaphore wait per instruction.
    Split every instruction with k>1 waits into (k-1) single-wait NoOps on
    the same engine immediately before it — identical blocking semantics."""
    import concourse.mybir as mybir

    n = 0
    for f in nc.m.functions:
        for bb in f.blocks:
            new_il = []
            for inst in bb.instructions:
                si = inst.sync_info
                waits = list(si.on_wait) if si is not None else []
                if len(waits) > 1:
                    for w in waits[:-1]:
                        nop = mybir.InstNoOp(name=f"{inst.name}-wsplit{n}", ins=[], outs=[])
                        n += 1
                        nop.engine = inst.engine
                        nop.sync_info = mybir.SyncInfo(on_wait=[w], on_update=[])
                        nc.register_instruction(nop, overwrite=True)
                        new_il.append(nop)
                    inst.sync_info = mybir.SyncInfo(
                        on_wait=[waits[-1]], on_update=list(si.on_update)
                    )
                new_il.append(inst)
            bb.instructions = new_il
    return n


def _build_nc(dve_pairs=(4, 8), skew=SKEW):
    import concourse.bass as bass
    import concourse.mybir as mybir
    import concourse.tile as tile

    f32 = mybir.dt.float32
    bf16 = mybir.dt.bfloat16
    fp16 = mybir.dt.float16
    fp8 = mybir.dt.float8e4

    DVE_SKEW = 9   # slots between a DVE pair's sq and its aggs

    nc = bass.Bass()
    c5_d = nc.dram_tensor("c5", [KROWS, B * N], fp8, kind="ExternalInput")
    p5_d = nc.dram_tensor("p5", [KROWS, N_JOBS * JM], fp8, kind="ExternalInput")
    ft_d = nc.dram_tensor("ft", [128, B * NT * F], fp16, kind="ExternalInput")
    wt_d = nc.dram_tensor("wt", [F, M * OUT_D], bf16, kind="ExternalInput")
    out_d = nc.dram_tensor("out", [N_JOBS * L_SLAB, OUT_D], f32, kind="ExternalOutput")

    # chunk pairs: sqs of chunks (2u, 2u+1) land in one 2-bank PSUM tile
    # [128, 1024] (cols 0:416 and 512:928); ONE strided elementwise op
    # computes both reciprocals (amortizes per-instruction overhead).
    # Most pairs: ACT Reciprocal LUT. Pairs u in dve_pairs of each job are
    # donated to DVE (copy to SBUF first so the PSUM frees fast, then the
    # exact 6-cpe reciprocal); their agg matmuls are deferred DVE_SKEW
    # slots, with start/stop flags tracked by emission order.
    # Job-PAIR weighters (M=32 rows, psum partition offsets 0/32) are
    # interleaved into later slots as PE filler work.
    with (
        nc.allow_low_precision(reason="split-fp8 matmul is ~4e-3-rel exact; verified vs oracle"),
        tile.TileContext(nc) as tc,
    ):
        with (
            tc.tile_pool(name="const", bufs=1) as cpool,
            tc.tile_pool(name="kt", bufs=DVE_SKEW + 3) as ktpool,
            tc.tile_pool(name="s32", bufs=2) as s32pool,
            tc.tile_pool(name="sq", bufs=3, space="PSUM") as sqpool,
            tc.tile_pool(name="acc", bufs=1, space="PSUM") as accpool,
        ):
            # split big input DMAs by batch elem and spread them across
            # engine queues so the first chunks land fast
            c5bs, ftbs = [], []
            for b in range(B):
                c5b = cpool.tile([KROWS, N], fp8, name=f"c5_{b}")
                c5bs.append(c5b)
                ftb = cpool.tile([128, NT * F], fp16, name=f"ft_{b}")
                ftbs.append(ftb)
            p5s = cpool.tile([KROWS, N_JOBS * JM], fp8)
            wts = cpool.tile([F, M * OUT_D], bf16)
            nc.sync.dma_start(c5bs[0][:], c5_d[:, 0:N])
            nc.scalar.dma_start(p5s[:], p5_d[:, :])
            nc.gpsimd.dma_start(ftbs[0][:], ft_d[:, 0 : NT * F])
            nc.sync.dma_start(c5bs[1][:], c5_d[:, N : 2 * N])
            nc.gpsimd.dma_start(ftbs[1][:], ft_d[:, NT * F : 2 * NT * F])
            nc.scalar.dma_start(wts[:], wt_d[:, :])

            # combined bf16 agg per job-pair: cols (m, j in pair, l')
            agg01 = cpool.tile([F, 2 * JM], bf16, name="agg01")
            agg23 = cpool.tile([F, 2 * JM], bf16, name="agg23")
            aggP = [agg01, agg23]
            # one shared weighter-out bank: rows (pair, j, l') = (jj, l')
            op = accpool.tile([2 * L_SLAB * 2, OUT_D], f32, tag="op", bufs=1, name="op")

            NPAIR = NT // 2                      # 16 pairs per job
            pairs = [(jj, u) for jj in range(N_JOBS) for u in range(NPAIR)]
            TOTP = len(pairs)
            kts = {}
            aggs = {}
            emitted = [0] * N_JOBS               # agg chunks emitted per job
            agg_slots = {}                       # slot -> list of pidx
            for pidx in range(TOTP):
                jj, u = pairs[pidx]
                s = pidx + (DVE_SKEW if u in dve_pairs else skew)
                agg_slots.setdefault(s, []).append(pidx)
            LAST = max(agg_slots)
            wq = []   # pending weighter-matmul closures (PE filler work)

            def emit_weighter(jp):
                # weighter for job pair jp (jobs 2*jp, 2*jp+1): M=32 rows
                for mi in range(M):
                    def mk(jp=jp, mi=mi):
                        nc.tensor.matmul(
                            op[jp * 32 : (jp + 1) * 32, :],
                            lhsT=aggP[jp][:, mi * 32 : (mi + 1) * 32],
                            rhs=wts[:, mi * OUT_D : (mi + 1) * OUT_D],
                            start=(mi == 0),
                            stop=(mi == M - 1),
                        )
                    wq.append(mk)

            for slot in range(LAST + 1):
                if slot < TOTP:
                    jj, u = pairs[slot]
                    b = jj // N_SLABS
                    sq = sqpool.tile([128, 1024], f32, tag="sq")
                    for half in range(2):
                        t = 2 * u + half
                        nc.tensor.matmul(
                            sq[:, half * 512 : half * 512 + JM],
                            lhsT=c5bs[b][:].rearrange("p (t x) -> p t x", t=NT)[:, t, :],
                            rhs=p5s[:, jj * JM : (jj + 1) * JM],
                            start=True,
                            stop=True,
                        )
                    kt = ktpool.tile([128, 2 * JM], fp16, tag="kt")
                    sq2v = sq[:].rearrange("p (i x) -> p i x", i=2)[:, :, 0:JM]
                    kt2v = kt[:].rearrange("p (i x) -> p i x", i=2)
                    if u in dve_pairs:
                        s32 = s32pool.tile([128, 2 * JM], f32, tag="s32")
                        s2v = s32[:].rearrange("p (i x) -> p i x", i=2)
                        nc.vector.tensor_copy(s2v, sq2v)
                        nc.vector.reciprocal(kt[:], s32[:])
                    else:
                        _act_reciprocal(nc, kt2v, sq2v)
                    kts[slot] = kt
                for p2 in agg_slots.get(slot, ()):
                    jj2, u2 = pairs[p2]
                    b2 = jj2 // N_SLABS
                    if emitted[jj2] == 0:
                        agg_tile = accpool.tile([F, JM], f32, tag="agg", bufs=1)
                        aggs[jj2] = agg_tile
                    for half in range(2):
                        t2 = 2 * u2 + half
                        nc.tensor.matmul(
                            aggs[jj2][:],
                            lhsT=ftbs[b2][:, (jj2 % N_SLABS * 0 + t2) * F : (t2 + 1) * F],
                            rhs=kts[p2][:, half * JM : (half + 1) * JM],
                            start=(emitted[jj2] == 0),
                            stop=(emitted[jj2] == NT - 1),
                        )
                        emitted[jj2] += 1
                    del kts[p2]
                    if emitted[jj2] == NT:
                        # bf16 piece of this job's agg into its pair slot:
                        # dst cols (m, j, l')
                        jp, j = jj2 // 2, jj2 % 2
                        dst = aggP[jp][:].rearrange(
                            "p (m j l) -> p m j l", m=M, j=2
                        )[:, :, j, :]
                        aggv = aggs[jj2][:].rearrange("p (m l) -> p m l", m=M)
                        nc.vector.tensor_copy(dst, aggv)
                        if j == 1:
                            emit_weighter(jp)
                # PE filler: up to 2 pending weighter matmuls per slot
                for _ in range(2):
                    if wq:
                        wq.pop(0)()
            while wq:
                wq.pop(0)()
            oS = cpool.tile([N_JOBS * L_SLAB, OUT_D], f32, name="oS")
            nc.vector.tensor_copy(oS[:], op[:])
            nc.sync.dma_start(out_d[:, :], oS[:])

    _split_multi_waits(nc)
    return nc


def _get_nc(dve_pairs=(4, 8), skew=SKEW):
    global _NC, _NC_KEY
    if _NC is None or _NC_KEY != (dve_pairs, skew):
        _NC = _build_nc(dve_pairs, skew)
        _NC_KEY = (dve_pairs, skew)
    return _NC


def _split_seq(x, n_pieces):
    """Sequential e4m3 split with escalating power-of-2 scales.
    Returns list of logical f64 pieces (each exactly e4m3*2^-g) summing
    to x up to a ~2^-4/piece-converging residual."""
    resid = np.asarray(x, np.float64).copy()
    pieces = []
    for _ in range(n_pieces):
        m = np.abs(resid).max()
        gamma = 1.0 if m == 0 else 2.0 ** np.floor(np.log2(224.0 / m))
        piece = (resid * gamma).astype(E4).astype(np.float64) / gamma
        pieces.append(piece)
        resid = resid - piece
    return pieces


def _balance_row(lhs_val, rhs_val):
    """Per-row power-of-2 balance: returns (e4m3(lhs*A), e4m3(rhs/A))."""
    lm = np.abs(lhs_val).max()
    rm = np.abs(rhs_val).max()
    if lm == 0 or rm == 0:
        A = 1.0
    else:
        A = 2.0 ** np.round(0.5 * (np.log2(rm) - np.log2(lm)))
        while lm * A > 224:
            A /= 2
        while rm / A > 224:
            A *= 2
    return (lhs_val * A).astype(E4), (rhs_val / A).astype(E4)


def _prep_all(points, centers, W_weighter):
    """Build all device inputs. Returns (c5, ft, wt, p5_list[8])."""
    coords = points[:, :, :D].astype(np.float64).reshape(B * N, D)   # [BN, 3]
    feats = points[:, :, D:].astype(np.float32)                      # [B, n, f]

    # probe columns, globally (all cores): [B, L, M, 3]
    probes = centers[:, :, None, :].astype(np.float64) + PROBES[None, None].astype(np.float64)
    pcols = probes.reshape(B * L * M, D)                             # [C, 3]

    q = 10.0 * (coords ** 2).sum(-1)                                 # [BN]
    r = 10.0 * (pcols ** 2).sum(-1) + 1.0                            # [C]
    t = -20.0 * pcols                                                # [C, 3]

    lhs_rows = []  # point side, e4m3 [BN]
    rhs_rows = []  # probe side, e4m3 [C]
    ones_c = np.ones_like(r)
    ones_n = np.ones_like(q)
    for piece in _split_seq(q, N_PIECES):
        l8, r8 = _balance_row(piece, ones_c)
        lhs_rows.append(l8)
        rhs_rows.append(r8)
    for piece in _split_seq(r, N_PIECES):
        l8, r8 = _balance_row(ones_n, piece)
        lhs_rows.append(l8)
        rhs_rows.append(r8)
    for k in range(D):
        cp = _split_seq(coords[:, k], N_PIECES)
        tp = _split_seq(t[:, k], N_PIECES)
        for i in range(N_PIECES):
            for j in range(N_PIECES):
                if i + j > MAX_IJ:
                    continue
                l8, r8 = _balance_row(cp[i], tp[j])
                lhs_rows.append(l8)
                rhs_rows.append(r8)
    assert len(lhs_rows) == N_ROWS
    while len(lhs_rows) < KROWS:  # pad to 128 rows (full-rate PE tile mode)
        lhs_rows.append(np.zeros_like(lhs_rows[0]))
        rhs_rows.append(np.zeros_like(rhs_rows[0]))

    c5 = np.ascontiguousarray(np.stack(lhs_rows))          # [KROWS, B*N]

    # probe side rows arranged per core: RHS [KROWS, C] with C=(B, L, M)
    RHS = np.stack(rhs_rows).reshape(KROWS, B, L, M)
    p5_list = []
    for core in range(N_CORES):
        p5 = np.zeros((KROWS, N_JOBS, M, L_SLAB), E4)
        for b in range(B):
            for sl in range(N_SLABS):
                jj = b * N_SLABS + sl
                lo = core * L_LOC + sl * L_SLAB
                p5[:, jj] = RHS[:, b, lo : lo + L_SLAB, :].transpose(0, 2, 1)
        p5_list.append(np.ascontiguousarray(p5).reshape(KROWS, N_JOBS * JM))

    # ft[p, (b, t, f)] = feats[b, t*128+p, f]   (fp16)
    ft = (
        np.ascontiguousarray(feats.reshape(B, NT, 128, F).transpose(2, 0, 1, 3))
        .reshape(128, B * NT * F)
        .astype(np.float16)
    )

    # wt[f, (piece, m, o)] = piece_{0,1} of W[(m*F+f), o] / n in bf16.
    wn = (
        np.ascontiguousarray(
            (W_weighter.astype(np.float64) / N).reshape(M, F, OUT_D).transpose(1, 0, 2)
        ).reshape(F, M * OUT_D)
    )
    wt = wn.astype(ml_dtypes.bfloat16)  # [F, M*OUT_D]
    return c5, ft, wt, p5_list


def kernel(points, centers, W_weighter, b_weighter):
    from concourse.bass_utils import run_bass_kernel_spmd

    points = np.asarray(points)
    centers = np.asarray(centers)
    W_weighter = np.asarray(W_weighter)
    b_weighter = np.asarray(b_weighter)

    nc = _get_nc()
    c5, ft, wt, p5_list = _prep_all(points, centers, W_weighter)
    in_maps = [
        {"c5": c5, "ft": ft, "p5": p5_list[core], "wt": wt}
        for core in range(N_CORES)
    ]
    res = run_bass_kernel_spmd(nc, in_maps, core_ids=list(range(N_CORES))).results

    out = np.empty((B, L, OUT_D), np.float32)
    for core in range(N_CORES):
        r = res[core]["out"]  # [(jj, l'), OUT_D]
        for jj in range(N_JOBS):
            b, s = jj // N_SLABS, jj % N_SLABS
            lo = core * L_LOC + s * L_SLAB
            out[b, lo : lo + L_SLAB] = r[jj * L_SLAB : (jj + 1) * L_SLAB]
    out += b_weighter.astype(np.float32)[None, None, :]
    return out


# revision 24
# speedup vs baseline: 1.1119x; 1.1119x over previous
"""Trainium2 Bass kernel for nn_CrossConvLayerV2 (gnn_message_passing).

Math (reference):
    coords = points[..., :3]; feats = points[..., 3:]          # [B,n,3], [B,n,f]
    probes[b,l,m] = centers[b,l] + PROBES[m]                    # [B,l,m,3]
    sq[b,l,m,n]  = ||coords[b,n] - probes[b,l,m]||^2
    kern         = C / (sq + C)          (C = 0.1)
    agg[b,l,m,f] = (1/n) sum_n kern * feats
    out[b,l,:]   = agg.reshape(l, m*f) @ W + bias               # [B,l,256]

Strategy (v2):
  - Shard centers dim l (256) over 8 cores -> 32 centers/core, zero
    communication; the host gathers the 8 [B,32,256] shards.
  - u = 10*sq + 1 via ONE fp8-e4m3 DoubleRow matmul (0.5 cyc/row, 2x
    the bf16 stream rate). The expansion u = q_n + r_col + sum_k c_k*t_k
    is split into 55 fp8 rows (5-piece sequential e4m3 splits with
    escalating power-of-2 scales; cross terms keep piece pairs with
    i+j<=4; per-row power-of-2 balance scales). fp8 x fp8 products are
    exact in fp32 PSUM accumulation, so u is accurate to ~4e-3 rel.
  - kern = 1/u: ACT Reciprocal LUT (~1.2e-5 rel) for most chunks, DVE
    exact reciprocal for t%8 < RECIP_DVE_OF8 chunks (engine balancing);
    written as fp16.
  - agg[f, (m,l')] += feats_chunk^T @ kern^T  (fp16, PSUM accumulate).
  - The PE instruction stream is software-pipelined with a skew so agg
    matmuls never stall on the reciprocal -> the PE p-state ramps to
    full clock (gaps reset the DVFS ramp).
  - Weighter: agg -> single bf16 piece; W/n in two bf16 pieces (hi+lo);
    52 small matmuls. b_weighter added on host (zeros here).
  - Walrus: at most ONE semaphore wait per instruction; a post-build
    pass splits multi-wait instructions into single-wait NoOp carriers.
"""

import sys

sys.path.insert(0, "/opt/trn_rl_repo")

import numpy as np
import ml_dtypes

# ---- problem constants (hardcoded per contract) ----
B, N, L, D, F = 2, 4096, 256, 3, 16
M = 26
OUT_D = 256
COEFF = 0.1
DIST = 3.0
N_CORES = 8
L_LOC = L // N_CORES          # 32 centers per core
N_SLABS = 2                   # jobs per batch elem per core
L_SLAB = L_LOC // N_SLABS     # 16 centers per job
JM = M * L_SLAB               # 416 = free dim of kern^T tiles
N_JOBS = B * N_SLABS          # 4 jobs per core
NT = N // 128                 # 32 n-chunks

N_PIECES = 5                  # e4m3 pieces per scalar quantity
MAX_IJ = 4                    # cross-term piece pairs kept: i+j <= MAX_IJ
N_ROWS = 2 * N_PIECES + 3 * sum(1 for i in range(N_PIECES) for j in range(N_PIECES) if i + j <= MAX_IJ)
# contraction rows padded to 128: the PE streams at FULL rate (1 col/cycle
# @2.4GHz) only when the stationary spans 128 partitions; small-K matmuls
# run at half rate. Zero rows are exact filler.
KROWS = 128

# of every 7 chunk-PAIRS, how many run the reciprocal on DVE (exact
# iterative divide, ~6 cycles/elem — ~6x ACT's LUT, but a parallel engine)
RECIP_DVE_OF7 = 0
# PE software-pipeline skew in PAIRS: aggs of pair u issue after sqs of pair u+SKEW
SKEW = 2

E4 = ml_dtypes.float8_e4m3


def _make_probes() -> np.ndarray:
    angles = np.array(
        [[j * 0.125 - 0.125, i * 0.125 + (j - 1) * 0.0625] for j in range(3) for i in range(8)]
        + [[-0.25, 0.0], [0.25, 0.0]],
        dtype=np.float64,
    ) * (2.0 * np.pi)
    a, b = angles[:, 0], angles[:, 1]
    pts = np.stack([np.sin(a), np.cos(a) * np.cos(b), np.cos(a) * np.sin(b)], axis=-1) * DIST
    return pts.astype(np.float32)  # [26, 3]


PROBES = _make_probes()

_NC = None
_NC_KEY = None


def _act_reciprocal(nc, out_ap, in_ap):
    """nc.scalar.activation(func=Reciprocal) minus the library guard.
    out = 1/in_ on the ACT engine (LUT path; measured ~1.2e-5 rel here)."""
    import concourse.mybir as mybir

    eng = nc.scalar
    inputs = [eng.lower_ap(in_ap)]
    for val in (0.0, 1.0, 0.0):  # bias, scale, alpha — immediates
        inputs.append(mybir.ImmediateValue(dtype=mybir.dt.float32, value=val))
    return eng.add_instruction(
        mybir.InstActivation(
            name=nc.get_next_instruction_name(),
            func=mybir.ActivationFunctionType.Reciprocal,
            ins=inputs,
            outs=[eng.lower_ap(out_ap)],
        )
    )


def _split_multi_waits(nc):
    """This walrus build encodes at most ONE semaphore wait per instruction.
    Split every instruction with k>1 waits into (k-1) single-wait NoOps on
    the same engine immediately before it — identical blocking semantics."""
    import concourse.mybir as mybir

    n = 0
    for f in nc.m.functions:
        for bb in f.blocks:
            new_il = []
            for inst in bb.instructions:
                si = inst.sync_info
                waits = list(si.on_wait) if si is not None else []
                if len(waits) > 1:
                    for w in waits[:-1]:
                        nop = mybir.InstNoOp(name=f"{inst.name}-wsplit{n}", ins=[], outs=[])
                        n += 1
                        nop.engine = inst.engine
                        nop.sync_info = mybir.SyncInfo(on_wait=[w], on_update=[])
                        nc.register_instruction(nop, overwrite=True)
                        new_il.append(nop)
                    inst.sync_info = mybir.SyncInfo(
                        on_wait=[waits[-1]], on_update=list(si.on_update)
                    )
                new_il.append(inst)
            bb.instructions = new_il
    return n


def _build_nc(groups_of=3, skew=SKEW):
    import concourse.bass as bass
    import concourse.mybir as mybir
    import concourse.tile as tile

    f32 = mybir.dt.float32
    bf16 = mybir.dt.bfloat16
    fp16 = mybir.dt.float16
    fp8 = mybir.dt.float8e4

    nc = bass.Bass()
    c5_d = nc.dram_tensor("c5", [KROWS, B * N], fp8, kind="ExternalInput")
    p5_d = nc.dram_tensor("p5", [KROWS, N_JOBS * JM], fp8, kind="ExternalInput")
    ft_d = nc.dram_tensor("ft", [128, B * NT * F], fp16, kind="ExternalInput")
    wt_d = nc.dram_tensor("wt", [F, M * OUT_D], bf16, kind="ExternalInput")
    out_d = nc.dram_tensor("out", [N_JOBS * L_SLAB, OUT_D], f32, kind="ExternalOutput")

    # chunk GROUPS (3,3,...,3,2 per job): sqs land in one 3-bank PSUM tile
    # (cols g*512 .. g*512+416); ONE strided ACT op computes the whole
    # group's reciprocals (amortizes per-instruction overhead).
    # Job-PAIR weighters (M=32 rows, psum partition offsets 0/32) are
    # interleaved into later slots as PE filler; each pair's [32, 256]
    # result is copied+DMA'd as soon as its weighter finishes.
    with (
        nc.allow_low_precision(reason="split-fp8 matmul is ~4e-3-rel exact; verified vs oracle"),
        tile.TileContext(nc) as tc,
    ):
        with (
            tc.tile_pool(name="const", bufs=1) as cpool,
            tc.tile_pool(name="kt", bufs=skew + 2) as ktpool,
            tc.tile_pool(name="sq", bufs=2, space="PSUM") as sqpool,
            tc.tile_pool(name="acc", bufs=1, space="PSUM") as accpool,
        ):
            # split big input DMAs (c5 by quarter, ft by batch) across
            # engine queues so the first chunks land fast
            NQ = 4
            c5qs = []
            for qq in range(NQ):
                c5q = cpool.tile([KROWS, (B * N) // NQ], fp8, name=f"c5_{qq}")
                c5qs.append(c5q)
            ftbs = []
            for b in range(B):
                ftb = cpool.tile([128, NT * F], fp16, name=f"ft_{b}")
                ftbs.append(ftb)
            p5s = cpool.tile([KROWS, N_JOBS * JM], fp8)
            wts = cpool.tile([F, M * OUT_D], bf16)
            QN = (B * N) // NQ
            nc.scalar.dma_start(p5s[:], p5_d[:, :])
            nc.sync.dma_start(c5qs[0][:], c5_d[:, 0:QN])
            nc.gpsimd.dma_start(ftbs[0][:], ft_d[:, 0 : NT * F])
            nc.sync.dma_start(c5qs[1][:], c5_d[:, QN : 2 * QN])
            nc.sync.dma_start(c5qs[2][:], c5_d[:, 2 * QN : 3 * QN])
            nc.gpsimd.dma_start(ftbs[1][:], ft_d[:, NT * F : 2 * NT * F])
            nc.sync.dma_start(c5qs[3][:], c5_d[:, 3 * QN : 4 * QN])
            nc.scalar.dma_start(wts[:], wt_d[:, :])

            def c5_ap(b, t):
                # chunk t of batch b lives in quarter qq at local chunk lt
                gchunk = b * NT + t
                per_q = (B * NT) // NQ
                qq, lt = divmod(gchunk, per_q)
                return c5qs[qq][:].rearrange("p (t x) -> p t x", t=per_q)[:, lt, :]

            # combined bf16 agg per job-pair: cols (m, j in pair, l')
            agg01 = cpool.tile([F, 2 * JM], bf16, name="agg01")
            agg23 = cpool.tile([F, 2 * JM], bf16, name="agg23")
            aggP = [agg01, agg23]
            # one shared weighter-out bank: rows (pair, j, l') = (jj, l')
            op = accpool.tile([2 * L_SLAB * 2, OUT_D], f32, tag="op", bufs=1, name="op")

            # groups of chunks per job: sizes 3..3,2 summing to NT
            groups = []
            for jj in range(N_JOBS):
                t0 = 0
                while t0 < NT:
                    cnt = min(groups_of, NT - t0)
                    if NT - t0 - cnt == 1:   # avoid a trailing 1-group
                        cnt -= 1
                    groups.append((jj, t0, cnt))
                    t0 += cnt
            TOTG = len(groups)
            kts = {}
            aggs = {}
            emitted = [0] * N_JOBS
            wq = []   # pending weighter-matmul closures (PE filler work)

            def emit_weighter(jp):
                # weighter for job pair jp (jobs 2*jp, 2*jp+1): M=32 rows
                for mi in range(M):
                    def mk(jp=jp, mi=mi):
                        nc.tensor.matmul(
                            op[jp * 32 : (jp + 1) * 32, :],
                            lhsT=aggP[jp][:, mi * 32 : (mi + 1) * 32],
                            rhs=wts[:, mi * OUT_D : (mi + 1) * OUT_D],
                            start=(mi == 0),
                            stop=(mi == M - 1),
                        )
                        if mi == M - 1:
                            oSp = cpool.tile([2 * L_SLAB, OUT_D], f32, name=f"oS_{jp}")
                            nc.vector.tensor_copy(oSp[:], op[jp * 32 : (jp + 1) * 32, :])
                            nc.sync.dma_start(
                                out_d[jp * 2 * L_SLAB : (jp + 1) * 2 * L_SLAB, :], oSp[:]
                            )
                    wq.append(mk)

            for slot in range(TOTG + skew):
                if slot < TOTG:
                    jj, t0, cnt = groups[slot]
                    b = jj // N_SLABS
                    sq = sqpool.tile([128, 512 * 3], f32, tag="sq")
                    for g in range(cnt):
                        t = t0 + g
                        nc.tensor.matmul(
                            sq[:, g * 512 : g * 512 + JM],
                            lhsT=c5_ap(b, t),
                            rhs=p5s[:, jj * JM : (jj + 1) * JM],
                            start=True,
                            stop=True,
                        )
                    kt = ktpool.tile([128, 3 * JM], fp16, tag="kt")
                    sqv = sq[:].rearrange("p (i x) -> p i x", i=3)[:, 0:cnt, 0:JM]
                    ktv = kt[:].rearrange("p (i x) -> p i x", i=3)[:, 0:cnt, :]
                    _act_reciprocal(nc, ktv, sqv)
                    kts[slot] = kt
                g2 = slot - skew
                if g2 >= 0 and g2 < TOTG:
                    jj2, t0g, cnt2 = groups[g2]
                    b2 = jj2 // N_SLABS
                    if emitted[jj2] == 0:
                        agg_tile = accpool.tile([F, JM], f32, tag="agg", bufs=1)
                        aggs[jj2] = agg_tile
                    for g in range(cnt2):
                        t2 = t0g + g
                        nc.tensor.matmul(
                            aggs[jj2][:],
                            lhsT=ftbs[b2][:, t2 * F : (t2 + 1) * F],
                            rhs=kts[g2][:, g * JM : (g + 1) * JM],
                            start=(emitted[jj2] == 0),
                            stop=(emitted[jj2] == NT - 1),
                        )
                        emitted[jj2] += 1
                    del kts[g2]
                    if emitted[jj2] == NT:
                        # bf16 piece of this job's agg into its pair slot:
                        # dst cols (m, j, l')
                        jp, j = jj2 // 2, jj2 % 2
                        dst = aggP[jp][:].rearrange(
                            "p (m j l) -> p m j l", m=M, j=2
                        )[:, :, j, :]
                        aggv = aggs[jj2][:].rearrange("p (m l) -> p m l", m=M)
                        nc.vector.tensor_copy(dst, aggv)
                        if j == 1:
                            emit_weighter(jp)
                # PE filler: up to 2 pending weighter matmuls per slot
                for _ in range(2):
                    if wq:
                        wq.pop(0)()
            while wq:
                wq.pop(0)()

    _split_multi_waits(nc)
    return nc


def _get_nc(groups_of=3, skew=SKEW):
    global _NC, _NC_KEY
    if _NC is None or _NC_KEY != (groups_of, skew):
        _NC = _build_nc(groups_of, skew)
        _NC_KEY = (groups_of, skew)
    return _NC


def _split_seq(x, n_pieces):
    """Sequential e4m3 split with escalating power-of-2 scales.
    Returns list of logical f64 pieces (each exactly e4m3*2^-g) summing
    to x up to a ~2^-4/piece-converging residual."""
    resid = np.asarray(x, np.float64).copy()
    pieces = []
    for _ in range(n_pieces):
        m = np.abs(resid).max()
        gamma = 1.0 if m == 0 else 2.0 ** np.floor(np.log2(224.0 / m))
        piece = (resid * gamma).astype(E4).astype(np.float64) / gamma
        pieces.append(piece)
        resid = resid - piece
    return pieces


def _balance_row(lhs_val, rhs_val):
    """Per-row power-of-2 balance: returns (e4m3(lhs*A), e4m3(rhs/A))."""
    lm = np.abs(lhs_val).max()
    rm = np.abs(rhs_val).max()
    if lm == 0 or rm == 0:
        A = 1.0
    else:
        A = 2.0 ** np.round(0.5 * (np.log2(rm) - np.log2(lm)))
        while lm * A > 224:
            A /= 2
        while rm / A > 224:
            A *= 2
    return (lhs_val * A).astype(E4), (rhs_val / A).astype(E4)


def _prep_all(points, centers, W_weighter):
    """Build all device inputs. Returns (c5, ft, wt, p5_list[8])."""
    coords = points[:, :, :D].astype(np.float64).reshape(B * N, D)   # [BN, 3]
    feats = points[:, :, D:].astype(np.float32)                      # [B, n, f]

    # probe columns, globally (all cores): [B, L, M, 3]
    probes = centers[:, :, None, :].astype(np.float64) + PROBES[None, None].astype(np.float64)
    pcols = probes.reshape(B * L * M, D)                             # [C, 3]

    q = 10.0 * (coords ** 2).sum(-1)                                 # [BN]
    r = 10.0 * (pcols ** 2).sum(-1) + 1.0                            # [C]
    t = -20.0 * pcols                                                # [C, 3]

    lhs_rows = []  # point side, e4m3 [BN]
    rhs_rows = []  # probe side, e4m3 [C]
    ones_c = np.ones_like(r)
    ones_n = np.ones_like(q)
    for piece in _split_seq(q, N_PIECES):
        l8, r8 = _balance_row(piece, ones_c)
        lhs_rows.append(l8)
        rhs_rows.append(r8)
    for piece in _split_seq(r, N_PIECES):
        l8, r8 = _balance_row(ones_n, piece)
        lhs_rows.append(l8)
        rhs_rows.append(r8)
    for k in range(D):
        cp = _split_seq(coords[:, k], N_PIECES)
        tp = _split_seq(t[:, k], N_PIECES)
        for i in range(N_PIECES):
            for j in range(N_PIECES):
                if i + j > MAX_IJ:
                    continue
                l8, r8 = _balance_row(cp[i], tp[j])
                lhs_rows.append(l8)
                rhs_rows.append(r8)
    assert len(lhs_rows) == N_ROWS
    while len(lhs_rows) < KROWS:  # pad to 128 rows (full-rate PE tile mode)
        lhs_rows.append(np.zeros_like(lhs_rows[0]))
        rhs_rows.append(np.zeros_like(rhs_rows[0]))

    c5 = np.ascontiguousarray(np.stack(lhs_rows))          # [KROWS, B*N]

    # probe side rows arranged per core: RHS [KROWS, C] with C=(B, L, M)
    RHS = np.stack(rhs_rows).reshape(KROWS, B, L, M)
    p5_list = []
    for core in range(N_CORES):
        p5 = np.zeros((KROWS, N_JOBS, M, L_SLAB), E4)
        for b in range(B):
            for sl in range(N_SLABS):
                jj = b * N_SLABS + sl
                lo = core * L_LOC + sl * L_SLAB
                p5[:, jj] = RHS[:, b, lo : lo + L_SLAB, :].transpose(0, 2, 1)
        p5_list.append(np.ascontiguousarray(p5).reshape(KROWS, N_JOBS * JM))

    # ft[p, (b, t, f)] = feats[b, t*128+p, f]   (fp16)
    ft = (
        np.ascontiguousarray(feats.reshape(B, NT, 128, F).transpose(2, 0, 1, 3))
        .reshape(128, B * NT * F)
        .astype(np.float16)
    )

    # wt[f, (piece, m, o)] = piece_{0,1} of W[(m*F+f), o] / n in bf16.
    wn = (
        np.ascontiguousarray(
            (W_weighter.astype(np.float64) / N).reshape(M, F, OUT_D).transpose(1, 0, 2)
        ).reshape(F, M * OUT_D)
    )
    wt = wn.astype(ml_dtypes.bfloat16)  # [F, M*OUT_D]
    return c5, ft, wt, p5_list


def kernel(points, centers, W_weighter, b_weighter):
    from concourse.bass_utils import run_bass_kernel_spmd

    points = np.asarray(points)
    centers = np.asarray(centers)
    W_weighter = np.asarray(W_weighter)
    b_weighter = np.asarray(b_weighter)

    nc = _get_nc()
    c5, ft, wt, p5_list = _prep_all(points, centers, W_weighter)
    in_maps = [
        {"c5": c5, "ft": ft, "p5": p5_list[core], "wt": wt}
        for core in range(N_CORES)
    ]
    res = run_bass_kernel_spmd(nc, in_maps, core_ids=list(range(N_CORES))).results

    out = np.empty((B, L, OUT_D), np.float32)
    for core in range(N_CORES):
        r = res[core]["out"]  # [(jj, l'), OUT_D]
        for jj in range(N_JOBS):
            b, s = jj // N_SLABS, jj % N_SLABS
            lo = core * L_LOC + s * L_SLAB
            out[b, lo : lo + L_SLAB] = r[jj * L_SLAB : (jj + 1) * L_SLAB]
    out += b_weighter.astype(np.float32)[None, None, :]
    return out
